# revision 20
# baseline (speedup 1.0000x reference)
"""Trainium2 Bass kernel for the GraphicalBranch GNN message-passing problem.

Math being computed (verified equivalent to the reference):
  - Per-sample graphs are fully connected WITH self-loops over the nc2=28
    pair-nodes, so segment_sum(x[src], dst) == broadcast of the per-sample
    row-sum S[b] = sum_r x[b, r, :].
  - The final key-matching gather h[rows] commutes with the row-wise linear
    layer, so we only run the W_self matmul on the 10 gathered rows per
    sample instead of all 28:
        out[b*10+k] = relu(xg[b*10+k] @ W_self + (S[b] @ W_nbr) + b)
  - rows are computed on host from slicing_tensor/object_pairs (pure index
    arithmetic) exactly as the reference's LUT does.

Sharding: data-parallel over samples; each of the 8 cores gets 128 samples
(3584 x-rows, 1280 output rows). Weights replicated.

Per-core device program (matmul operands bf16, f32 PSUM accumulate):
  1. S = G^T @ x on TensorE. x streams in 4 chunks of 896 rows (= exactly
     32 samples), so the same tiny one-hot block G[j][p, s] =
     ((j*128+p)//28 == s), s in [0,32), works for every chunk; chunk ch
     accumulates into PSUM partitions [32ch, 32ch+32) via
     tile_position=(0, 32ch). W_self matmuls for output tiles 0-3 are
     interleaved between chunks to fill PE gaps (their PSUM groups stay
     open until A is ready).
  2. Transpose S via 4 PE transposes -> S^T tiles (bf16).
  3. A = S @ W_nbr + b via 4 accumulating matmuls plus a K=1 ones-matmul
     that adds b to every row of the PSUM accumulator.
  4. Per output tile: 4 matmuls xg @ W_self (lhsT = xgT slices) + one
     expansion matmul E_t @ A (E[r, s] = 1 iff s == r//10) closing the
     same PSUM accumulation group; ReLU on ScalarE; stores in pairs.

All DRAM inputs are host-prelaid so every load is a plain contiguous
[128, F] DMA (one descriptor per partition). Loads are split across the
two HWDGE rings: sync carries the x stream, scalar carries everything
else, ordered by when each tensor is first needed.
"""

import numpy as np
import ml_dtypes

# ---- problem constants (hardcoded; kernel.py must be self-contained) ----
B = 1024          # samples
NOBJ = 8          # objects per sample
NC2 = 28          # pair-nodes per sample
MAXR = 10         # relations per sample
D = 512           # feature dim
NCORES = 8
BL = B // NCORES          # 128 samples per core
RL = BL * NC2             # 3584 x-rows per core
ML = BL * MAXR            # 1280 output rows per core
KT = D // 128             # 4 contraction tiles
MT = ML // 128            # 10 output row tiles per core
RT = RL // 128            # 28 x row-tiles per core
XCH = 4                   # x chunks (896 rows = 32 samples each)
RJ = RT // XCH            # 7 row-tiles per chunk
SW = BL // XCH            # 32 samples per chunk

BF16 = ml_dtypes.bfloat16

_compiled = None


def _build_bass():
    import concourse.bacc as bacc
    import concourse.bass as bass
    import concourse.mybir as mybir
    from concourse import tile

    f32 = mybir.dt.float32
    bf16 = mybir.dt.bfloat16

    nc = bacc.Bacc("TRN2", target_bir_lowering=False, debug=False,
                   num_devices=NCORES)

    # all inputs prelaid on host: partition-major, contiguous free dim
    x_d = nc.dram_tensor("x", [XCH, 128, RJ * D], bf16, kind="ExternalInput")
    g_d = nc.dram_tensor("g", [128, RJ * SW], bf16, kind="ExternalInput")
    xgT_d = nc.dram_tensor("xgT", [128, KT * ML], bf16, kind="ExternalInput")
    ws_d = nc.dram_tensor("ws", [128, KT * D], bf16, kind="ExternalInput")
    wn_d = nc.dram_tensor("wn", [128, KT * D], bf16, kind="ExternalInput")
    eT_d = nc.dram_tensor("eT", [128, ML], bf16, kind="ExternalInput")
    b_d = nc.dram_tensor("bias", [1, D], bf16, kind="ExternalInput")
    id_d = nc.dram_tensor("ident", [128, 128], bf16, kind="ExternalInput")
    out_d = nc.dram_tensor("out", [ML, D], f32, kind="ExternalOutput")

    with tile.TileContext(nc) as tc:
        with (
            tc.tile_pool(name="const", bufs=1) as cpool,
            tc.tile_pool(name="x", bufs=4) as xpool,
            tc.tile_pool(name="outp", bufs=3) as opool,
            tc.tile_pool(name="psum", bufs=4, space=bass.MemorySpace.PSUM) as ppool,
            tc.tile_pool(name="psumS", bufs=1, space=bass.MemorySpace.PSUM) as pspool,
            tc.tile_pool(name="psumT", bufs=2, space=bass.MemorySpace.PSUM) as ptpool,
            tc.tile_pool(name="psumA", bufs=1, space=bass.MemorySpace.PSUM) as papool,
        ):
            # ---- scalar-ring loads, ordered by first need ----
            g_sb = cpool.tile([128, RJ, SW], bf16)
            nc.scalar.dma_start(g_sb[:], g_d.rearrange("p (j s) -> p j s", s=SW))
            ws_sb = cpool.tile([128, KT, D], bf16)
            nc.scalar.dma_start(ws_sb[:], ws_d.rearrange("p (t n) -> p t n", n=D))
            xgT_sb = cpool.tile([128, KT, ML], bf16)
            nc.scalar.dma_start(xgT_sb[:], xgT_d.rearrange("p (t m) -> p t m", m=ML))
            id_sb = cpool.tile([128, 128], bf16)
            nc.scalar.dma_start(id_sb[:], id_d[:, :])
            wn_sb = cpool.tile([128, KT, D], bf16)
            nc.scalar.dma_start(wn_sb[:], wn_d.rearrange("p (t n) -> p t n", n=D))
            b_sb = cpool.tile([1, D], bf16)
            nc.scalar.dma_start(b_sb[:], b_d[:, :])
            eT_sb = cpool.tile([128, ML], bf16)
            nc.scalar.dma_start(eT_sb[:], eT_d[:, :])
            ones_sb = cpool.tile([1, 128], bf16)
            nc.gpsimd.memset(ones_sb[:], 1.0)
            ones512 = cpool.tile([1, D], bf16)
            nc.gpsimd.memset(ones512[:], 1.0)
            # PE warmup: K=1 matmuls from memset tiles (no DMA dependency)
            # engage the HAM clock-gate before the first real matmul
            psW = ppool.tile([128, D], f32, tag="ps")
            for _ in range(12):
                nc.tensor.matmul(psW[:], ones_sb[:], ones512[:],
                                 start=True, stop=True)

            # ---- S accumulation, interleaved with early W_self groups ----
            psS = pspool.tile([128, D], f32)
            main_ps = {}

            def open_main_group(t):
                ps = ppool.tile([128, D], f32, tag="ps")
                for kt in range(KT):
                    nc.tensor.matmul(
                        ps[:],
                        xgT_sb[:, kt, t * 128:(t + 1) * 128],
                        ws_sb[:, kt, :],
                        start=(kt == 0), stop=False,
                    )
                main_ps[t] = ps

            for ch in range(XCH):
                xch = xpool.tile([128, RJ, D], bf16, tag="x")
                nc.sync.dma_start(xch[:], x_d[ch].rearrange("p (j d) -> p j d", d=D))
                for j in range(RJ):
                    nc.tensor.matmul(psS[ch * SW:(ch + 1) * SW, :],
                                     g_sb[:, j, :], xch[:, j, :],
                                     start=(j == 0), stop=(j == RJ - 1),
                                     tile_position=(0, ch * SW))
                open_main_group(ch)   # fill PE while next chunk streams

            s_nat = cpool.tile([128, D], bf16)
            nc.scalar.copy(s_nat[:], psS[:])

            # ---- transpose S -> S^T (bf16) ----
            s_bf = cpool.tile([128, KT, BL], bf16)
            for dt in range(KT):
                psT = ptpool.tile([128, BL], bf16, tag="psT")
                nc.tensor.transpose(psT[:], s_nat[:, dt * 128:(dt + 1) * 128],
                                    id_sb[:])
                nc.vector.tensor_copy(s_bf[:, dt, :], psT[:])

            # ---- A = S @ W_nbr + b (bias via K=1 ones matmul) ----
            psA = papool.tile([128, D], f32)
            for kt in range(KT):
                nc.tensor.matmul(psA[:], s_bf[:, kt, :], wn_sb[:, kt, :],
                                 start=(kt == 0), stop=False)
            nc.tensor.matmul(psA[:], ones_sb[:], b_sb[:],
                             start=False, stop=True)
            a_bf = cpool.tile([128, D], bf16)
            nc.vector.tensor_copy(a_bf[:], psA[:])

            # ---- close groups / remaining tiles; stores in pairs ----
            out_r = out_d.rearrange("(t u p) n -> t p u n", p=128, u=2)
            ot = None
            for t in range(MT):
                if t not in main_ps:
                    open_main_group(t)
                ps = main_ps.pop(t)
                nc.tensor.matmul(ps[:], eT_sb[:, t * 128:(t + 1) * 128],
                                 a_bf[:], start=False, stop=True)
                if t % 2 == 0:
                    ot = opool.tile([128, 2, D], f32, tag="ot")
                if t % 2 == 0:
                    nc.scalar.activation(ot[:, 0, :], ps[:],
                                         mybir.ActivationFunctionType.Relu)
                else:
                    nc.vector.tensor_relu(ot[:, 1, :], ps[:])
                    nc.sync.dma_start(out_r[t // 2], ot[:])

    nc.compile()
    return nc


def _get_compiled():
    global _compiled
    if _compiled is None:
        _compiled = _build_bass()
    return _compiled


def _host_prep(inputs):
    """Shard + preprocess on host. Returns per-core input maps."""
    x = np.asarray(inputs["spatial_branch_feature_map"], dtype=np.float32)
    W_self = np.asarray(inputs["W_self"], dtype=np.float32)
    W_nbr = np.asarray(inputs["W_nbr"], dtype=np.float32)
    b = np.asarray(inputs["b"], dtype=np.float32)
    st = np.asarray(inputs["slicing_tensor"])
    op = np.asarray(inputs["object_pairs"])

    N = x.shape[0]
    n = NOBJ
    # exact replication of the reference's LUT-based row computation
    keys = st[:, 0].astype(np.int64) * (n * n) + st[:, 1].astype(np.int64) * n \
        + st[:, 2].astype(np.int64)
    lut = np.zeros(B * n * n, dtype=np.int64)
    lut[keys] = np.arange(N, dtype=np.int64)
    pmin = np.minimum(op[..., 0], op[..., 1]).astype(np.int64)
    pmax = np.maximum(op[..., 0], op[..., 1]).astype(np.int64)
    rel_keys = (np.arange(B, dtype=np.int64)[:, None] * (n * n)
                + pmin * n + pmax).reshape(-1)
    rows = lut[rel_keys]                      # [B*MAXR] global row index

    xg = x[rows]                              # [B*MAXR, D]
    # x: [NCORES, XCH, 128, RJ*D]; sbuf[p, j, :] = x_core[ch*896 + j*128 + p]
    x_bf = np.ascontiguousarray(
        x.astype(BF16).reshape(NCORES, XCH, RJ, 128, D)
        .transpose(0, 1, 3, 2, 4).reshape(NCORES, XCH, 128, RJ * D))
    # xgT: [NCORES, 128, KT*ML]; sbuf[p, kt, m] = xg_core[m, kt*128+p]
    xgT = np.ascontiguousarray(
        xg.astype(BF16).reshape(NCORES, ML, KT, 128)
        .transpose(0, 3, 2, 1).reshape(NCORES, 128, KT * ML))

    def wlay(W):  # [D, D] -> [128, KT*D]: sbuf[p, kt, n] = W[kt*128+p, n]
        return np.ascontiguousarray(
            W.astype(BF16).reshape(KT, 128, D).transpose(1, 0, 2)
            .reshape(128, KT * D))

    ws = wlay(W_self)
    wn = wlay(W_nbr)
    eT = (np.arange(ML)[None, :] // MAXR
          == np.arange(128)[:, None]).astype(BF16)   # [128, ML]
    # shared one-hot block: g[p, j*SW + s] = ((j*128 + p)//NC2 == s)
    jj = np.arange(RJ * 128)
    g = (jj[:, None] // NC2 == np.arange(SW)[None, :]).astype(BF16)
    g = np.ascontiguousarray(
        g.reshape(RJ, 128, SW).transpose(1, 0, 2).reshape(128, RJ * SW))
    bias = b.astype(BF16).reshape(1, D)
    ident = np.eye(128, dtype=BF16)

    in_maps = []
    for c in range(NCORES):
        in_maps.append({
            "x": x_bf[c], "xgT": xgT[c], "g": g,
            "ws": ws, "wn": wn, "eT": eT, "bias": bias, "ident": ident,
        })
    return in_maps


def run(inputs, trace=False):
    """Returns (full_output, BassKernelResults)."""
    from concourse.bass_utils import run_bass_kernel_spmd

    nc = _get_compiled()
    in_maps = _host_prep(inputs)
    res = run_bass_kernel_spmd(nc, in_maps, core_ids=list(range(NCORES)),
                               trace=trace)
    out = np.concatenate([r["out"] for r in res.results], axis=0)
    return out, res


def kernel(**inputs) -> np.ndarray:
    out, _ = run(inputs, trace=False)
    return out


# revision 21
# speedup vs baseline: 1.0899x; 1.0899x over previous
"""Trainium2 Bass kernel for the GraphicalBranch GNN message-passing problem.

Math being computed (verified equivalent to the reference):
  - Per-sample graphs are fully connected WITH self-loops over the nc2=28
    pair-nodes, so segment_sum(x[src], dst) == broadcast of the per-sample
    row-sum S[b] = sum_r x[b, r, :].
  - The final key-matching gather h[rows] commutes with the row-wise linear
    layer, so we only run the W_self matmul on the 10 gathered rows per
    sample instead of all 28:
        out[b*10+k] = relu(xg[b*10+k] @ W_self + (S[b] @ W_nbr) + b)
  - rows are computed on host from slicing_tensor/object_pairs (pure index
    arithmetic) exactly as the reference's LUT does.

Sharding: data-parallel over samples; each of the 8 cores gets 128 samples
(3584 x-rows, 1280 output rows). Weights replicated.

Per-core device program (matmul operands bf16, f32 PSUM accumulate):
  1. S = G^T @ x on TensorE. x streams in 4 chunks of 896 rows (= exactly
     32 samples), so the same tiny one-hot block G[j][p, s] =
     ((j*128+p)//28 == s), s in [0,32), works for every chunk; chunk ch
     accumulates into PSUM partitions [32ch, 32ch+32) via
     tile_position=(0, 32ch). W_self matmuls for output tiles 0-3 are
     interleaved between chunks to fill PE gaps (their PSUM groups stay
     open until A is ready).
  2. Transpose S via 4 PE transposes -> S^T tiles (bf16).
  3. A = S @ W_nbr + b via 4 accumulating matmuls plus a K=1 ones-matmul
     that adds b to every row of the PSUM accumulator.
  4. Per output tile: 4 matmuls xg @ W_self (lhsT = xgT slices) + one
     expansion matmul E_t @ A (E[r, s] = 1 iff s == r//10) closing the
     same PSUM accumulation group; ReLU on ScalarE; stores in pairs.

All DRAM inputs are host-prelaid so every load is a plain contiguous
[128, F] DMA (one descriptor per partition). Loads are split across the
two HWDGE rings: sync carries the x stream, scalar carries everything
else, ordered by when each tensor is first needed.
"""

import numpy as np
import ml_dtypes

# ---- problem constants (hardcoded; kernel.py must be self-contained) ----
B = 1024          # samples
NOBJ = 8          # objects per sample
NC2 = 28          # pair-nodes per sample
MAXR = 10         # relations per sample
D = 512           # feature dim
NCORES = 8
BL = B // NCORES          # 128 samples per core
RL = BL * NC2             # 3584 x-rows per core
ML = BL * MAXR            # 1280 output rows per core
KT = D // 128             # 4 contraction tiles
MT = ML // 128            # 10 output row tiles per core
RT = RL // 128            # 28 x row-tiles per core
XCH = 4                   # x chunks (896 rows = 32 samples each)
RJ = RT // XCH            # 7 row-tiles per chunk
SW = BL // XCH            # 32 samples per chunk

BF16 = ml_dtypes.bfloat16

_compiled = None


def _build_bass():
    import concourse.bacc as bacc
    import concourse.bass as bass
    import concourse.mybir as mybir
    from concourse import tile

    f32 = mybir.dt.float32
    bf16 = mybir.dt.bfloat16

    nc = bacc.Bacc("TRN2", target_bir_lowering=False, debug=False,
                   num_devices=NCORES)

    # all inputs prelaid on host: partition-major, contiguous free dim
    x_d = nc.dram_tensor("x", [XCH, 128, RJ * D], bf16, kind="ExternalInput")
    g_d = nc.dram_tensor("g", [128, RJ * SW], bf16, kind="ExternalInput")
    xgT_d = nc.dram_tensor("xgT", [128, KT * ML], bf16, kind="ExternalInput")
    ws_d = nc.dram_tensor("ws", [128, KT * D], bf16, kind="ExternalInput")
    wn_d = nc.dram_tensor("wn", [128, KT * D], bf16, kind="ExternalInput")
    eT_d = nc.dram_tensor("eT", [128, ML], bf16, kind="ExternalInput")
    b_d = nc.dram_tensor("bias", [1, D], bf16, kind="ExternalInput")
    id_d = nc.dram_tensor("ident", [128, 128], bf16, kind="ExternalInput")
    out_d = nc.dram_tensor("out", [ML, D], f32, kind="ExternalOutput")

    with tile.TileContext(nc) as tc:
        with (
            tc.tile_pool(name="const", bufs=1) as cpool,
            tc.tile_pool(name="x", bufs=4) as xpool,
            tc.tile_pool(name="outp", bufs=3) as opool,
            tc.tile_pool(name="psum", bufs=4, space=bass.MemorySpace.PSUM) as ppool,
            tc.tile_pool(name="psumS", bufs=1, space=bass.MemorySpace.PSUM) as pspool,
            tc.tile_pool(name="psumT", bufs=2, space=bass.MemorySpace.PSUM) as ptpool,
            tc.tile_pool(name="psumA", bufs=1, space=bass.MemorySpace.PSUM) as papool,
        ):
            # ---- loads: sync ring carries x0, xgT, ws, x1..x3 in
            # ---- consumption order; scalar ring carries the small/tail set
            g_sb = cpool.tile([128, RJ, SW], bf16)
            nc.scalar.dma_start(g_sb[:], g_d.rearrange("p (j s) -> p j s", s=SW))
            wn_sb = cpool.tile([128, KT, D], bf16)
            nc.scalar.dma_start(wn_sb[:], wn_d.rearrange("p (t n) -> p t n", n=D))
            id_sb = cpool.tile([128, 128], bf16)
            nc.scalar.dma_start(id_sb[:], id_d[:, :])
            b_sb = cpool.tile([1, D], bf16)
            nc.scalar.dma_start(b_sb[:], b_d[:, :])
            eT_sb = cpool.tile([128, ML], bf16)
            nc.scalar.dma_start(eT_sb[:], eT_d[:, :])
            ones_sb = cpool.tile([1, 128], bf16)
            nc.gpsimd.memset(ones_sb[:], 1.0)

            # ---- S accumulation, interleaved with early W_self groups ----
            psS = pspool.tile([128, D], f32)
            main_ps = {}

            def open_main_group(t):
                ps = ppool.tile([128, D], f32, tag="ps")
                for kt in range(KT):
                    nc.tensor.matmul(
                        ps[:],
                        xgT_sb[:, kt, t * 128:(t + 1) * 128],
                        ws_sb[:, kt, :],
                        start=(kt == 0), stop=False,
                    )
                main_ps[t] = ps

            for ch in range(XCH):
                xch = xpool.tile([128, RJ, D], bf16, tag="x")
                nc.sync.dma_start(xch[:], x_d[ch].rearrange("p (j d) -> p j d", d=D))
                if ch == 0:
                    xgT_sb = cpool.tile([128, KT, ML], bf16)
                    nc.sync.dma_start(
                        xgT_sb[:], xgT_d.rearrange("p (t m) -> p t m", m=ML))
                    ws_sb = cpool.tile([128, KT, D], bf16)
                    nc.sync.dma_start(
                        ws_sb[:], ws_d.rearrange("p (t n) -> p t n", n=D))
                for j in range(RJ):
                    nc.tensor.matmul(psS[ch * SW:(ch + 1) * SW, :],
                                     g_sb[:, j, :], xch[:, j, :],
                                     start=(j == 0), stop=(j == RJ - 1),
                                     tile_position=(0, ch * SW))
                open_main_group(ch)   # fill PE while next chunk streams

            s_nat = cpool.tile([128, D], bf16)
            nc.scalar.copy(s_nat[:], psS[:])

            # ---- transpose S -> S^T (bf16) ----
            s_bf = cpool.tile([128, KT, BL], bf16)
            for dt in range(KT):
                psT = ptpool.tile([128, BL], bf16, tag="psT")
                nc.tensor.transpose(psT[:], s_nat[:, dt * 128:(dt + 1) * 128],
                                    id_sb[:])
                nc.vector.tensor_copy(s_bf[:, dt, :], psT[:])

            # ---- A = S @ W_nbr + b (bias via K=1 ones matmul) ----
            psA = papool.tile([128, D], f32)
            for kt in range(KT):
                nc.tensor.matmul(psA[:], s_bf[:, kt, :], wn_sb[:, kt, :],
                                 start=(kt == 0), stop=False)
            nc.tensor.matmul(psA[:], ones_sb[:], b_sb[:],
                             start=False, stop=True)
            a_bf = cpool.tile([128, D], bf16)
            nc.vector.tensor_copy(a_bf[:], psA[:])

            # ---- close groups / remaining tiles; stores in pairs ----
            out_r = out_d.rearrange("(t u p) n -> t p u n", p=128, u=2)
            ot = None
            for t in range(MT):
                if t not in main_ps:
                    open_main_group(t)
                ps = main_ps.pop(t)
                nc.tensor.matmul(ps[:], eT_sb[:, t * 128:(t + 1) * 128],
                                 a_bf[:], start=False, stop=True)
                if t % 2 == 0:
                    ot = opool.tile([128, 2, D], f32, tag="ot")
                nc.scalar.activation(ot[:, t % 2, :], ps[:],
                                     mybir.ActivationFunctionType.Relu)
                if t % 2 == 1:
                    nc.sync.dma_start(out_r[t // 2], ot[:])

    nc.compile()
    return nc


def _get_compiled():
    global _compiled
    if _compiled is None:
        _compiled = _build_bass()
    return _compiled


def _host_prep(inputs):
    """Shard + preprocess on host. Returns per-core input maps."""
    x = np.asarray(inputs["spatial_branch_feature_map"], dtype=np.float32)
    W_self = np.asarray(inputs["W_self"], dtype=np.float32)
    W_nbr = np.asarray(inputs["W_nbr"], dtype=np.float32)
    b = np.asarray(inputs["b"], dtype=np.float32)
    st = np.asarray(inputs["slicing_tensor"])
    op = np.asarray(inputs["object_pairs"])

    N = x.shape[0]
    n = NOBJ
    # exact replication of the reference's LUT-based row computation
    keys = st[:, 0].astype(np.int64) * (n * n) + st[:, 1].astype(np.int64) * n \
        + st[:, 2].astype(np.int64)
    lut = np.zeros(B * n * n, dtype=np.int64)
    lut[keys] = np.arange(N, dtype=np.int64)
    pmin = np.minimum(op[..., 0], op[..., 1]).astype(np.int64)
    pmax = np.maximum(op[..., 0], op[..., 1]).astype(np.int64)
    rel_keys = (np.arange(B, dtype=np.int64)[:, None] * (n * n)
                + pmin * n + pmax).reshape(-1)
    rows = lut[rel_keys]                      # [B*MAXR] global row index

    xg = x[rows]                              # [B*MAXR, D]
    # x: [NCORES, XCH, 128, RJ*D]; sbuf[p, j, :] = x_core[ch*896 + j*128 + p]
    x_bf = np.ascontiguousarray(
        x.astype(BF16).reshape(NCORES, XCH, RJ, 128, D)
        .transpose(0, 1, 3, 2, 4).reshape(NCORES, XCH, 128, RJ * D))
    # xgT: [NCORES, 128, KT*ML]; sbuf[p, kt, m] = xg_core[m, kt*128+p]
    xgT = np.ascontiguousarray(
        xg.astype(BF16).reshape(NCORES, ML, KT, 128)
        .transpose(0, 3, 2, 1).reshape(NCORES, 128, KT * ML))

    def wlay(W):  # [D, D] -> [128, KT*D]: sbuf[p, kt, n] = W[kt*128+p, n]
        return np.ascontiguousarray(
            W.astype(BF16).reshape(KT, 128, D).transpose(1, 0, 2)
            .reshape(128, KT * D))

    ws = wlay(W_self)
    wn = wlay(W_nbr)
    eT = (np.arange(ML)[None, :] // MAXR
          == np.arange(128)[:, None]).astype(BF16)   # [128, ML]
    # shared one-hot block: g[p, j*SW + s] = ((j*128 + p)//NC2 == s)
    jj = np.arange(RJ * 128)
    g = (jj[:, None] // NC2 == np.arange(SW)[None, :]).astype(BF16)
    g = np.ascontiguousarray(
        g.reshape(RJ, 128, SW).transpose(1, 0, 2).reshape(128, RJ * SW))
    bias = b.astype(BF16).reshape(1, D)
    ident = np.eye(128, dtype=BF16)

    in_maps = []
    for c in range(NCORES):
        in_maps.append({
            "x": x_bf[c], "xgT": xgT[c], "g": g,
            "ws": ws, "wn": wn, "eT": eT, "bias": bias, "ident": ident,
        })
    return in_maps


def run(inputs, trace=False):
    """Returns (full_output, BassKernelResults)."""
    from concourse.bass_utils import run_bass_kernel_spmd

    nc = _get_compiled()
    in_maps = _host_prep(inputs)
    res = run_bass_kernel_spmd(nc, in_maps, core_ids=list(range(NCORES)),
                               trace=trace)
    out = np.concatenate([r["out"] for r in res.results], axis=0)
    return out, res


def kernel(**inputs) -> np.ndarray:
    out, _ = run(inputs, trace=False)
    return out
